# revision 11
# baseline (speedup 1.0000x reference)
"""Multi-head self-attention (CogView PB-relax variant) on 8 TRN2 NeuronCores.

Problem: B=2, S=2048, D=1024, H=16 heads, Dh=64.
  q/k/v = hidden @ W{q,k,v}.T + b          (per-head slices)
  scores = (q k^T + attn_bias) / 8 + (1-mask)*(-BIG)
  out    = softmax(scores) @ v             (PB-relax softmax == plain softmax)

Sharding: tensor-parallel over heads. Core c owns heads (2c, 2c+1) for both
batch rows: it reads full hidden, W-row slices [128c:128c+128], bias slice
[h=2c:2c+2], and writes output channels [128c:128(c+1)].

Per-core algorithm (all on-chip, flash-style, no DRAM spill):
  1. hidden^T via PE transposes; q^T,k^T,v^T projections with f32r matmuls.
  2. scores computed TRANSPOSED, tile [k=128, q=512] in PSUM:
     - bias arrives in natural [q,k] layout (contiguous DMA) and is
       transpose-accumulated into the PSUM tile by 4 PE transpose ops;
     - the q k^T matmul (contraction=64, both heads packed in the 128-row
       PE array via tile_position) accumulates on top.
  3. exp on ACT reads PSUM directly: out = exp(in*0.125 + maskbias[k]);
     maskbias is a per-partition column (k is the partition dim), so the
     attention mask is applied for free.
  4. ctx^T accumulated over k-chunks: lhsT = [v | 1] (65 cols) so row 64
     yields the masked softmax denominator; rhs = exp tile.
  5. epilogue: divide by denominator, PE-transpose back to [q, d], DMA out.
"""

import numpy as np

import concourse.bass as bass
import concourse.mybir as mybir
import concourse.tile as tile
from concourse import bacc, bass_utils
from concourse.masks import make_identity

F32 = mybir.dt.float32
F32R = mybir.dt.float32r
I32 = mybir.dt.int32
Exp = mybir.ActivationFunctionType.Exp

B, S, D = 2, 2048, 1024
NCORES = 8
HPC = 2            # heads per core
OC = HPC * 64      # 128 output channels per core
QB = 512           # q block (free dim of score tiles)
NQB = S // QB      # 4
NKC = S // 128     # 16 k-chunks per batch row
NSB = (B * S) // 512   # 8 token blocks for projections
NDC = D // 128     # 8 contraction chunks

MASK_NEG = -30000.0
SCALE = 0.125


def _r(ap):
    return ap.bitcast(F32R)


def _build_program():
    nc = bacc.Bacc(
        "TRN2", target_bir_lowering=False, debug=False, num_devices=NCORES
    )
    hidden = nc.dram_tensor("hidden_state", [B, S, D], F32, kind="ExternalInput").ap()
    amask = nc.dram_tensor("attention_mask", [B, S], I32, kind="ExternalInput").ap()
    abias = nc.dram_tensor("attention_bias", [HPC, S, S], F32, kind="ExternalInput").ap()
    wq = nc.dram_tensor("wq", [OC, D], F32, kind="ExternalInput").ap()
    bq = nc.dram_tensor("bq", [OC], F32, kind="ExternalInput").ap()
    wk = nc.dram_tensor("wk", [OC, D], F32, kind="ExternalInput").ap()
    bk = nc.dram_tensor("bk", [OC], F32, kind="ExternalInput").ap()
    wv = nc.dram_tensor("wv", [OC, D], F32, kind="ExternalInput").ap()
    bv = nc.dram_tensor("bv", [OC], F32, kind="ExternalInput").ap()
    out = nc.dram_tensor("out", [B, S, OC], F32, kind="ExternalOutput").ap()

    with tile.TileContext(nc) as tc:
        _attention(tc, out, hidden, amask, abias,
                   [wq, wk, wv], [bq, bk, bv])

    nc.compile()
    return nc


def _attention(tc, out, hidden, amask, abias, ws, bs):
    nc = tc.nc
    hflat = hidden.flatten_outer_dims()          # [4096, 1024]

    with tc.tile_pool(name="singles", bufs=1) as singles:
        ident = singles.tile([128, 128], F32)
        make_identity(nc, ident)

        # --- mask -> additive bias column layout [128, B, NKC] ------------
        mi = singles.tile([128, B, NKC], I32)
        nc.sync.dma_start(out=mi, in_=amask.rearrange("b (c p) -> p b c", p=128))
        mf = singles.tile([128, B, NKC], F32)
        nc.vector.tensor_copy(out=mf, in_=mi)
        mb = singles.tile([128, B, NKC], F32)
        nc.vector.tensor_scalar(
            out=mb, in0=mf, scalar1=-MASK_NEG, scalar2=MASK_NEG,
            op0=mybir.AluOpType.mult, op1=mybir.AluOpType.add,
        )

        # --- projection bias vectors [128, 1] -----------------------------
        bvec = []
        for i, b_ap in enumerate(bs):
            t = singles.tile([128, 1], F32, tag=f"bvec{i}")
            nc.sync.dma_start(out=t, in_=b_ap.rearrange("(p o) -> p o", o=1))
            bvec.append(t)

        # --- W^T [d-part, out-free], 8 chunks each ------------------------
        wt = []
        with tc.tile_pool(name="w_nat", bufs=2) as wnp, \
             tc.tile_pool(name="w_ps", bufs=4, space="PSUM") as wps:
            for i, w_ap in enumerate(ws):
                wn = wnp.tile([128, D], F32)
                nc.sync.dma_start(out=wn, in_=w_ap)
                t = singles.tile([128, D], F32R, tag=f"wt{i}")
                for dc in range(NDC):
                    blk = wps.tile([128, 128], F32)
                    nc.tensor.transpose(
                        out=blk, in_=wn[:, dc * 128:(dc + 1) * 128],
                        identity=ident)
                    nc.vector.tensor_copy(
                        out=t[:, dc * 128:(dc + 1) * 128], in_=blk)
                wt.append(t)

        # --- persistent activations --------------------------------------
        qt2 = singles.tile([128, B * S], F32R, tag="qt2")   # q^T, heads stacked
        kt2 = singles.tile([128, B * S], F32R, tag="kt2")   # k^T, heads stacked
        # v natural + ones column: [k-local, chunk, 66*head]
        va = singles.tile([128, 2 * NKC, 2 * 66], F32R, tag="va")
        ones_col = singles.tile([128, 1], F32)
        nc.vector.memset(ones_col, 1.0)

        # ============ phase 1: hidden^T + projections =====================
        with tc.tile_pool(name="h_nat", bufs=6) as hnp, \
             tc.tile_pool(name="h_t", bufs=2) as htp, \
             tc.tile_pool(name="v_t", bufs=2) as vtp, \
             tc.tile_pool(name="t_ps", bufs=4, space="PSUM") as tps, \
             tc.tile_pool(name="p_ps", bufs=3, space="PSUM") as pps:
            for sb in range(NSB):
                hts = htp.tile([128, NDC, 512], F32R)
                for i in range(4):
                    hn = hnp.tile([128, D], F32)
                    nc.sync.dma_start(
                        out=hn, in_=hflat[sb * 512 + i * 128:
                                          sb * 512 + (i + 1) * 128, :])
                    for dc in range(NDC):
                        blk = tps.tile([128, 128], F32)
                        nc.tensor.transpose(
                            out=blk, in_=hn[:, dc * 128:(dc + 1) * 128],
                            identity=ident)
                        nc.vector.tensor_copy(
                            out=hts[:, dc, i * 128:(i + 1) * 128], in_=blk)
                for w in range(3):
                    pp = pps.tile([128, 512], F32)
                    for dc in range(NDC):
                        nc.tensor.matmul(
                            out=pp,
                            lhsT=wt[w][:, dc * 128:(dc + 1) * 128],
                            rhs=hts[:, dc, :],
                            start=(dc == 0), stop=(dc == NDC - 1))
                    if w < 2:
                        dst = (qt2 if w == 0 else kt2)[:, sb * 512:(sb + 1) * 512]
                        nc.vector.tensor_scalar_add(
                            out=dst, in0=pp, scalar1=bvec[w])
                    else:
                        vt = vtp.tile([128, 512], F32)
                        nc.vector.tensor_scalar_add(out=vt, in0=pp, scalar1=bvec[2])
                        for i in range(4):
                            blk = tps.tile([128, 128], F32)
                            nc.tensor.transpose(
                                out=blk, in_=vt[:, i * 128:(i + 1) * 128],
                                identity=ident)
                            kb = sb * 4 + i
                            for h in range(HPC):
                                nc.vector.tensor_copy(
                                    out=va[:, kb, h * 66:h * 66 + 64],
                                    in_=blk[:, h * 64:(h + 1) * 64])
                                nc.vector.tensor_copy(
                                    out=va[:, kb, h * 66 + 64:h * 66 + 65],
                                    in_=ones_col)

        # ============ phase 2: attention ==================================
        with tc.tile_pool(name="biasq", bufs=6) as bqp, \
             tc.tile_pool(name="pt", bufs=6) as ptp, \
             tc.tile_pool(name="stage", bufs=3) as stp, \
             tc.tile_pool(name="small", bufs=4) as smp, \
             tc.tile_pool(name="osb", bufs=3) as osp, \
             tc.tile_pool(name="sc_ps", bufs=3, space="PSUM") as scp, \
             tc.tile_pool(name="ctx_ps", bufs=4, space="PSUM") as cxp:
            for qb in range(NQB):
                ctx = [[cxp.tile([65, QB], F32, tag="ctx", name=f"ctx{b}{h}")
                        for h in range(HPC)] for b in range(B)]
                # stream bias in [q-part, qsub, k] natural tiles, 1MB per DMA
                bt = {}
                for h in range(HPC):
                    for kg in range(4):   # groups of 4 k-chunks
                        t = bqp.tile([128, 4, 512], F32, tag="biasq")
                        nc.sync.dma_start(
                            out=t,
                            in_=abias[h, qb * QB:(qb + 1) * QB,
                                      kg * 512:(kg + 1) * 512]
                            .rearrange("(i p) k -> p i k", p=128))
                        bt[(h, kg)] = t
                for kc in range(NKC):
                    kg, ko = kc // 4, (kc % 4) * 128
                    for b in range(B):
                        pts = []
                        for h in range(HPC):
                            sc = scp.tile([128, QB], F32, tag="sc")
                            for i in range(4):
                                # i==0 clears the whole PSUM zero-region; the
                                # rest must not re-clear it (start=False writes
                                # fresh elements, has_written still 0 there)
                                nc.tensor.matmul(
                                    out=sc[:, i * 128:(i + 1) * 128],
                                    lhsT=bt[(h, kg)][:, i, ko:ko + 128],
                                    rhs=ident,
                                    is_transpose=True,
                                    start=(i == 0), stop=False,
                                    skip_group_check=True)
                            nc.tensor.matmul(
                                out=sc,
                                lhsT=kt2[h * 64:(h + 1) * 64,
                                         b * S + kc * 128:
                                         b * S + (kc + 1) * 128],
                                rhs=qt2[h * 64:(h + 1) * 64,
                                        b * S + qb * QB:
                                        b * S + (qb + 1) * QB],
                                start=False, stop=True,
                                tile_position=(h * 64, 0),
                                skip_group_check=True)
                            pt = ptp.tile([128, QB], F32R, tag="pt")
                            nc.scalar.activation(
                                out=pt, in_=sc, func=Exp,
                                bias=mb[:, b, kc:kc + 1], scale=SCALE)
                            pts.append(pt)
                        for h in range(HPC):
                            nc.tensor.matmul(
                                out=ctx[b][h],
                                lhsT=va[:, b * NKC + kc,
                                        h * 66:h * 66 + 65],
                                rhs=pts[h],
                                start=(kc == 0), stop=(kc == NKC - 1))
                # ---- epilogue: normalize, transpose to [q, d], store -----
                for b in range(B):
                    stage = stp.tile([128, QB], F32, tag="stage")
                    rst = stp.tile([128, QB], F32, tag="rst")
                    for h in range(HPC):
                        nc.vector.tensor_copy(
                            out=stage[h * 64:(h + 1) * 64, :],
                            in_=ctx[b][h][0:64, :])
                        # reciprocal rows land at 32-aligned partitions (0, 32)
                        nc.vector.reciprocal(
                            out=rst[32 * h:32 * h + 1, :],
                            in_=ctx[b][h][64:65, :])
                    osb = osp.tile([128, 4, 128], F32, tag="osb")
                    for i in range(4):
                        tp = scp.tile([128, 128], F32, tag="sc", name="ep_t")
                        rp = scp.tile([128, 128], F32, tag="sc", name="ep_r")
                        nc.tensor.transpose(
                            out=tp, in_=stage[:, i * 128:(i + 1) * 128],
                            identity=ident)
                        nc.tensor.transpose(
                            out=rp, in_=rst[:, i * 128:(i + 1) * 128],
                            identity=ident)
                        for h in range(HPC):
                            nc.vector.tensor_scalar_mul(
                                out=osb[:, i, h * 64:(h + 1) * 64],
                                in0=tp[:, h * 64:(h + 1) * 64],
                                scalar1=rp[:, 32 * h:32 * h + 1])
                    nc.sync.dma_start(
                        out=out[b, qb * QB:(qb + 1) * QB, :]
                        .rearrange("(i p) k -> p i k", p=128),
                        in_=osb)


_CACHE = {}


def _get_program():
    if "nc" not in _CACHE:
        _CACHE["nc"] = _build_program()
    return _CACHE["nc"]


def _shard_inputs(inputs):
    hs = np.ascontiguousarray(np.asarray(inputs["hidden_state"], dtype=np.float32))
    am = np.ascontiguousarray(np.asarray(inputs["attention_mask"], dtype=np.int32))
    ab = np.asarray(inputs["attention_bias"], dtype=np.float32)
    ws = {k: np.asarray(inputs[k], dtype=np.float32) for k in ("Wq", "Wk", "Wv")}
    vb = {k: np.asarray(inputs[k], dtype=np.float32) for k in ("bq", "bk", "bv")}
    in_maps = []
    for c in range(NCORES):
        r0, r1 = c * OC, (c + 1) * OC
        in_maps.append({
            "hidden_state": hs,
            "attention_mask": am,
            "attention_bias": np.ascontiguousarray(ab[0, HPC * c:HPC * (c + 1)]),
            "wq": np.ascontiguousarray(ws["Wq"][r0:r1]),
            "bq": np.ascontiguousarray(vb["bq"][r0:r1]),
            "wk": np.ascontiguousarray(ws["Wk"][r0:r1]),
            "bk": np.ascontiguousarray(vb["bk"][r0:r1]),
            "wv": np.ascontiguousarray(ws["Wv"][r0:r1]),
            "bv": np.ascontiguousarray(vb["bv"][r0:r1]),
        })
    return in_maps


def kernel(**inputs):
    nc = _get_program()
    in_maps = _shard_inputs(inputs)
    res = bass_utils.run_bass_kernel_spmd(
        nc, in_maps, core_ids=list(range(NCORES)))
    parts = [np.asarray(res.results[c]["out"]) for c in range(NCORES)]
    return np.concatenate(parts, axis=-1)


def run_profiled(inputs, trace=True):
    """test.py helper: returns (output, BassKernelResults)."""
    nc = _get_program()
    in_maps = _shard_inputs(inputs)
    res = bass_utils.run_bass_kernel_spmd(
        nc, in_maps, core_ids=list(range(NCORES)), trace=trace)
    parts = [np.asarray(res.results[c]["out"]) for c in range(NCORES)]
    return np.concatenate(parts, axis=-1), res
